# revision 4
# baseline (speedup 1.0000x reference)
"""Self-contained Trainium2 Bass kernel for a 2-layer GAT (PyG GATConv semantics).

Strategy (8 NeuronCores, SPMD), v2:
  - dst-node partitioning: core c owns global nodes [c*SH, (c+1)*SH), degree-
    sorted within the core. Nodes processed in batches of 128 (one node per
    SBUF partition); per-batch slot counts padded to a cross-core-uniform
    schedule.
  - per layer: node-sharded matmul produces the augmented node table
    [h | alpha_src | alpha_dst] (192-float rows = 768B, %256B for dma_gather),
    AllGather of the table, then an edge phase that gathers h[src]|as[src]
    rows with batched dma_gather (int16 indices; the 100352-row table is
    covered by TWO signed-base views of <=65536 rows each) and reduces
    sum(w * h[src]) per dst node with strided vector reduces.
  - slack slots (padding of the per-batch rectangle) gather an arbitrary row
    and are zeroed by a host-provided mask multiplied into the edge weights.
  - softmax without segment_max (shift cancels in num/den), and
    exp(leaky_relu(x, 0.2)) = max(exp(x), exp(0.2 x)), as in v1.
"""

import numpy as np
from contextlib import ExitStack


# ---------------------------------------------------------------- config

class Cfg:
    def __init__(self, N, E, SH):
        self.N = N
        self.E = E
        self.SH = SH
        self.NCORES = 8
        self.NPAD = 8 * SH
        self.PB = 128
        self.NB = SH // 128
        self.F = 128
        self.H1, self.C1 = 4, 32
        self.HSPLIT = 34816          # src rows < HSPLIT -> view A, else view B
        self.BASE_A = 2048           # view A base row (idx = src - 2048)
        self.BASE_B = 67584          # view B base row (idx = src - 67584)
        self.MAXC = 7                # max slot-columns per dma_gather call
        assert 7 * SH <= N < 8 * SH
        assert SH % 128 == 0


FULL = Cfg(N=100000, E=1600000, SH=12544)


# ---------------------------------------------------------------- host prep

def host_prep(cfg, edge_index):
    """Degree-sort nodes per core, build the uniform 2-view batch schedule,
    the per-core int16 gather index arrays (call layout) and the slack mask."""
    N, SH, PB, NB, NC = cfg.N, cfg.SH, cfg.PB, cfg.NB, cfg.NCORES
    src = np.concatenate([edge_index[0], np.arange(N, dtype=np.int64)])
    dst = np.concatenate([edge_index[1], np.arange(N, dtype=np.int64)])
    core_of = dst // SH

    perms = np.empty((NC, SH), np.int64)
    invs = np.empty((NC, SH), np.int64)
    edges = []
    for c in range(NC):
        m = core_of == c
        d_loc = dst[m] - c * SH
        deg = np.bincount(d_loc, minlength=SH)
        perm = np.argsort(-deg, kind="stable")
        perms[c] = perm
        invs[c, perm] = np.arange(SH)
        edges.append((d_loc, src[m]))

    # per-core edge arrays in pi space, split by view
    per_core = []
    for c in range(NC):
        d_loc, s_glb = edges[c]
        pos = invs[c, d_loc]                      # dst pi-local row
        c2 = s_glb // SH
        src_pi = c2 * SH + invs[c2, s_glb - c2 * SH]
        view = (src_pi >= cfg.HSPLIT).astype(np.int64)
        per_core.append((pos, src_pi, view))

    # per (batch, view) max degree over cores -> uniform schedule
    Dm = np.zeros((2, NB), np.int64)
    percore_cnt = []
    for c in range(NC):
        pos, src_pi, view = per_core[c]
        cnts = np.zeros((2, SH), np.int64)
        np.add.at(cnts, (view, pos), 1)
        percore_cnt.append(cnts)
        for v in range(2):
            m = cnts[v].reshape(NB, PB).max(axis=1)
            Dm[v] = np.maximum(Dm[v], m)
    colsv = Dm + 1                                # +1 slack col

    # call plan (shared across cores): per batch, per view, calls of <=MAXC cols
    # call = (batch, view, col0_in_batch, ncols, idx_off_cols)
    # Every call's final position (dst 127, last col) is RESERVED as slack so
    # the ucode's trailing-negative trim can never fire (idx there is >= 0).
    # Reserving costs dst 127 one slot per call; bump colsv until the lowest-
    # degree dst of every batch still fits its real edges.
    maxdeg127 = np.zeros((2, NB), np.int64)
    for c in range(NC):
        cnts = percore_cnt[c]
        for v in range(2):
            m = cnts[v].reshape(NB, PB)[:, PB - 1]
            maxdeg127[v] = np.maximum(maxdeg127[v], m)
    for v in range(2):
        for b in range(NB):
            while int(colsv[v][b]) - int(np.ceil(colsv[v][b] / cfg.MAXC)) < int(maxdeg127[v][b]):
                colsv[v][b] += 1
    D = colsv[0] + colsv[1]
    offs = np.zeros(NB + 1, np.int64)
    np.cumsum(D, out=offs[1:])
    S = int(offs[-1])

    calls = []
    idx_cols = 0
    finals = [[], []]  # per (view): list of (b, final_col_in_batch)
    for b in range(NB):
        for v in range(2):
            cv = int(colsv[v][b])
            c0 = 0 if v == 0 else int(colsv[0][b])
            x = 0
            while x < cv:
                nc_ = min(cfg.MAXC, cv - x)
                calls.append((b, v, c0 + x, nc_, idx_cols))
                finals[v].append((b, c0 + x + nc_ - 1))
                idx_cols += nc_ * 8
                x += nc_
    # per (b, v): set of reserved final cols (batch-local col ids)
    fin_by_bv = {}
    for v in range(2):
        for (b, fc) in finals[v]:
            fin_by_bv.setdefault((b, v), set()).add(fc)

    # per-core idx16 [128, S] in pi-slot space + mask
    idxbufs, masks = [], []
    base = np.array([cfg.BASE_A, cfg.BASE_B], np.int64)
    # slack targets: view A -> row BASE_A (idx 0, masked), view B -> row N
    # (idx N-BASE_B >= 0; row N is a -1e30 pad row, also masked)
    slackidx = np.array([0, N - cfg.BASE_B], np.int64)
    for c in range(NC):
        pos, src_pi, view = per_core[c]
        idx16 = np.empty((PB, S), np.int16)
        mask = np.ones((PB, S), np.float32)
        # default = slack
        for b in range(NB):
            o = int(offs[b])
            idx16[:, o:o + colsv[0][b]] = slackidx[0]
            mask[:, o:o + colsv[0][b]] = 0.0
            o2 = o + int(colsv[0][b])
            idx16[:, o2:o2 + colsv[1][b]] = slackidx[1]
            mask[:, o2:o2 + colsv[1][b]] = 0.0
        # place real edges: per (view, pos) slot order
        order = np.lexsort((src_pi, pos + SH * view))
        pv = (pos + SH * view)[order]
        first = np.searchsorted(pv, pv)
        slot = np.arange(len(pv)) - first
        poso = pos[order]
        viewo = view[order]
        srco = src_pi[order]
        b_of = poso // PB
        p_of = poso % PB
        # for dst row 127, remap slot index to skip reserved final columns
        slot_eff = slot.copy()
        m127 = p_of == PB - 1
        if m127.any():
            idxs127 = np.nonzero(m127)[0]
            for i in idxs127:
                b, v = int(b_of[i]), int(viewo[i])
                fins = fin_by_bv.get((b, v), ())
                base_c = 0 if v == 0 else int(colsv[0][b])
                s = int(slot[i])
                # s-th non-reserved column within this view range
                cc = 0
                k = -1
                while True:
                    k += 1
                    if (base_c + k) in fins:
                        continue
                    if cc == s:
                        break
                    cc += 1
                slot_eff[i] = k
        col = np.where(viewo == 0, slot_eff, colsv[0][b_of] + slot_eff)
        cpos = offs[b_of] + col
        idx16[p_of, cpos] = (srco - base[viewo]).astype(np.int16)
        mask[p_of, cpos] = 1.0
        # build call-layout idx buffer [128, idx_cols]
        ibuf = np.empty((PB, idx_cols), np.int16)
        for (b, v, c0, nc_, io) in calls:
            ni = PB * nc_
            flat = idx16[:, offs[b] + c0: offs[b] + c0 + nc_]  # [128, nc]
            flat = flat.T.reshape(ni)                          # i = col*128 + p
            arr = np.empty((PB, nc_ * 8), np.int16)
            j = np.arange(nc_ * 8)
            for p16 in range(16):
                arr[p16::16, :] = flat[j * 16 + p16][None, :]
            ibuf[:, io:io + nc_ * 8] = arr
        idxbufs.append(ibuf)
        masks.append(mask)

    sched = dict(Dm=Dm, colsv=colsv, D=D, offs=offs[:-1], S=S, calls=calls,
                 idx_cols=idx_cols)
    return perms, sched, idxbufs, masks


def make_wcats(cfg, W1, a_src1, a_dst1, W2, a_src2, a_dst2):
    F, H1, C1 = cfg.F, cfg.H1, cfg.C1
    W1T = np.ascontiguousarray(W1.T, dtype=np.float32)
    Bs1 = np.einsum("hck,hc->kh", W1.reshape(H1, C1, F), a_src1)
    Bd1 = np.einsum("hck,hc->kh", W1.reshape(H1, C1, F), a_dst1)
    wcat1 = np.concatenate([W1T, Bs1, Bd1], 1).astype(np.float32)  # [128,136]
    W2T = np.ascontiguousarray(W2.T, dtype=np.float32)
    Bs2 = (W2.T @ a_src2[0])[:, None]
    Bd2 = (W2.T @ a_dst2[0])[:, None]
    wcat2 = np.concatenate([W2T, Bs2, Bd2], 1).astype(np.float32)  # [128,130]
    return wcat1, wcat2


def make_core_inputs(cfg, x, perms, idxbufs, masks, wcat1, wcat2, b1):
    N, SH, NC = cfg.N, cfg.SH, cfg.NCORES
    maps = []
    b1_bcast = np.broadcast_to(b1.astype(np.float32), (128, 128)).copy()
    for c in range(NC):
        base = c * SH
        cnt = min(SH, N - base)
        perm = perms[c]
        valid = perm < cnt
        xs = np.zeros((SH, cfg.F), np.float32)
        xs[valid] = x[base + perm[valid]]
        p1 = np.zeros((SH, 4), np.float32)
        p1[~valid] = -1e30
        p2 = np.zeros((SH, 1), np.float32)
        p2[~valid] = -1e30
        maps.append({
            "x_shard": xs,
            "idxbuf": np.ascontiguousarray(idxbufs[c]),
            "mask": np.ascontiguousarray(masks[c]),
            "wcat1": wcat1, "wcat2": wcat2,
            "patch1": p1, "patch2": p2,
            "bias1": b1_bcast,
        })
    return maps


# ---------------------------------------------------------------- bass program

def split_multi_waits(nc):
    """Move all but the last wait of any multi-wait instruction onto
    same-engine NoOps (walrus accepts only one embedded wait)."""
    import concourse.mybir as mybir
    import bass_rust
    n_split = 0
    for f in nc.m.functions:
        for bb in f.blocks:
            lst = bb.instructions
            i = 0
            while i < len(lst):
                inst = lst[i]
                si = inst.sync_info
                if si is not None and len(si.on_wait) > 1:
                    waits = list(si.on_wait)
                    for k, w in enumerate(waits[:-1]):
                        nop = mybir.InstNoOp(name=f"{inst.name}-w{k}", ins=[], outs=[])
                        nop.engine = inst.engine
                        nop.sync_info = bass_rust.SyncInfo(on_wait=[w], on_update=[])
                        lst.insert(i, nop)
                        i += 1
                    inst.sync_info = bass_rust.SyncInfo(
                        on_wait=[waits[-1]], on_update=list(si.on_update))
                    n_split += 1
                i += 1
    return n_split


def build_bass(cfg, sched, split=True, stages=4, reps=1, nqueues=4):
    import concourse.bass as bass
    import concourse.mybir as mybir
    import concourse.tile as tile
    from concourse.masks import make_identity
    from concourse.library_config import mlp
    from concourse.library_overlay import lower_extended_insts

    fp = mybir.dt.float32
    i16 = mybir.dt.int16
    SH, NB, NPAD = cfg.SH, cfg.NB, cfg.NPAD
    AG_GROUPS = [list(range(cfg.NCORES))]
    Dm, colsv, Dtot = sched["Dm"], sched["colsv"], sched["D"]
    offs, S, calls, idx_cols = sched["offs"], sched["S"], sched["calls"], sched["idx_cols"]
    # per-batch call lists
    calls_by_b = [[] for _ in range(NB)]
    for (b, v, c0, nc_, io) in calls:
        calls_by_b[b].append((v, c0, nc_, io))
    W = 192  # table row floats (768B)

    nc = bass.Bass(num_swdge_queues=nqueues)
    x_shard = nc.declare_dram_parameter("x_shard", [SH, 128], fp, isOutput=False)
    idxbuf_d = nc.declare_dram_parameter("idxbuf", [128, idx_cols], i16, isOutput=False)
    mask_d = nc.declare_dram_parameter("mask", [128, S], fp, isOutput=False)
    wcat1_d = nc.declare_dram_parameter("wcat1", [128, 136], fp, isOutput=False)
    wcat2_d = nc.declare_dram_parameter("wcat2", [128, 130], fp, isOutput=False)
    patch1_d = nc.declare_dram_parameter("patch1", [SH, 4], fp, isOutput=False)
    patch2_d = nc.declare_dram_parameter("patch2", [SH, 1], fp, isOutput=False)
    bias1_d = nc.declare_dram_parameter("bias1", [128, 128], fp, isOutput=False)
    out_d = nc.declare_dram_parameter("out", [SH, 128], fp, isOutput=True)

    haug1_loc = nc.dram_tensor("haug1_loc", [SH, W], fp)
    ad1_loc = nc.dram_tensor("ad1_loc", [SH, 4], fp)
    x2_loc = nc.dram_tensor("x2_loc", [SH, 128], fp)
    haug2_loc = nc.dram_tensor("haug2_loc", [SH, W], fp)
    ad2_loc = nc.dram_tensor("ad2_loc", [SH, 1], fp)
    haug1_tab = nc.dram_tensor("haug1_tab", [NPAD, W], fp, addr_space="Shared")
    haug2_tab = nc.dram_tensor("haug2_tab", [NPAD, W], fp, addr_space="Shared")

    def vap(t, free_dims):
        a = t[tuple([slice(None)] * len(t.shape))]
        return bass.AP(tensor=a.tensor, offset=a.offset, ap=[a.ap[0]] + free_dims)

    with tile.TileContext(nc) as tc, ExitStack() as ctx:
        nc.gpsimd.load_library(mlp)
        ni_regs = {c: nc.gpsimd.to_reg(128 * c) for c in range(1, cfg.MAXC + 1)}

        consts = ctx.enter_context(tc.tile_pool(name="consts", bufs=1))
        ident = consts.tile([128, 128], fp)
        make_identity(nc, ident[:])
        warm_ps = ctx.enter_context(tc.tile_pool(name="warm_ps", bufs=2, space="PSUM"))

        def pe_sync():
            pe_warm = warm_ps.tile([128, 1], fp, space="PSUM", tag="pe_warm")
            nc.tensor.matmul(out=pe_warm[:], lhsT=ident[:], rhs=ident[:, 0:1],
                             start=True, stop=True)

        pe_sync()
        wc1_dma = consts.tile([128, 136], fp)
        nc.sync.dma_start(out=wc1_dma[:], in_=wcat1_d[:, :])
        wc1_sb = consts.tile([128, 136], fp)
        nc.vector.tensor_copy(out=wc1_sb[:], in_=wc1_dma[:])
        wc2_dma = consts.tile([128, 130], fp)
        nc.sync.dma_start(out=wc2_dma[:], in_=wcat2_d[:, :])
        wc2_sb = consts.tile([128, 130], fp)
        nc.vector.tensor_copy(out=wc2_sb[:], in_=wc2_dma[:])
        b1_sb = consts.tile([128, 128], fp)
        nc.sync.dma_start(out=b1_sb[:], in_=bias1_d[:, :])
        mask_sb = consts.tile([128, S], fp)
        nc.sync.dma_start(out=mask_sb[:], in_=mask_d[:, :])

        mm_x = ctx.enter_context(tc.tile_pool(name="mm_x", bufs=3))
        mm_ps = ctx.enter_context(tc.tile_pool(name="mm_ps", bufs=2, space="PSUM"))
        mm_st = ctx.enter_context(tc.tile_pool(name="mm_st", bufs=3))

        def matmul_phase(src_dram, wc_sb, ncols, patch_dram, pw, haug_dram, hw,
                         ad_dram, elu_in: bool):
            for t in range(NB):
                r0 = t * 128
                x_t = mm_x.tile([128, 128], fp, tag="x_t")
                nc.sync.dma_start(out=x_t[:], in_=src_dram[r0:r0 + 128, :])
                if elu_in:
                    z = mm_x.tile([128, 128], fp, tag="z")
                    nc.vector.tensor_tensor(out=z[:], in0=x_t[:], in1=b1_sb[:],
                                            op=mybir.AluOpType.add)
                    nc.vector.tensor_scalar_max(x_t[:], z[:], 0.0)
                    nc.vector.tensor_scalar_min(z[:], z[:], 0.0)
                    nc.scalar.activation(z[:], z[:], mybir.ActivationFunctionType.Exp)
                    nc.vector.tensor_tensor(out=x_t[:], in0=x_t[:], in1=z[:],
                                            op=mybir.AluOpType.add)
                    nc.vector.tensor_scalar_add(x_t[:], x_t[:], -1.0)
                xt_ps = mm_ps.tile([128, 128], fp, space="PSUM", tag="xt_ps")
                nc.tensor.transpose(out=xt_ps[:], in_=x_t[:], identity=ident[:])
                xt_sb = mm_x.tile([128, 128], fp, tag="xt_sb")
                nc.vector.tensor_copy(out=xt_sb[:], in_=xt_ps[:])
                o_ps = mm_ps.tile([128, ncols], fp, space="PSUM", tag="o_ps")
                nc.tensor.matmul(out=o_ps[:], lhsT=xt_sb[:], rhs=wc_sb[:, :ncols],
                                 start=True, stop=True)
                st = mm_st.tile([128, ncols], fp, tag="st")
                nc.vector.tensor_copy(out=st[:], in_=o_ps[:])
                pt = mm_st.tile([128, pw], fp, tag="pt")
                nc.sync.dma_start(out=pt[:], in_=patch_dram[r0:r0 + 128, :])
                nc.vector.tensor_tensor(out=st[:, 128:128 + pw],
                                        in0=st[:, 128:128 + pw], in1=pt[:],
                                        op=mybir.AluOpType.add)
                nc.sync.dma_start(out=haug_dram[r0:r0 + 128, 0:hw], in_=st[:, 0:hw])
                nc.sync.dma_start(out=ad_dram[r0:r0 + 128, :],
                                  in_=st[:, 128 + pw:128 + 2 * pw])

        eg_g = ctx.enter_context(tc.tile_pool(name="eg_g", bufs=3))
        eg_i = ctx.enter_context(tc.tile_pool(name="eg_i", bufs=3))
        eg_w = ctx.enter_context(tc.tile_pool(name="eg_w", bufs=3))
        eg_m = ctx.enter_context(tc.tile_pool(name="eg_m", bufs=2))
        eg_s = ctx.enter_context(tc.tile_pool(name="eg_s", bufs=3))
        eg_o = ctx.enter_context(tc.tile_pool(name="eg_o", bufs=3))

        qrr = [0]

        def edge_phase(haug_tab, ad_dram, H, out_dram, hw):
            Cc = 128 // H
            viewA = haug_tab[cfg.BASE_A:cfg.BASE_A + 65536, :]
            viewB = haug_tab[cfg.BASE_B:cfg.NPAD, :]
            views = (viewA, viewB)
            for b in range(NB):
                d = int(Dtot[b])
                o = int(offs[b])
                r0 = b * 128
                bcalls = calls_by_b[b]
                icol0 = bcalls[0][3]
                icoln = bcalls[-1][3] + bcalls[-1][2] * 8
                ad_t = eg_s.tile([128, H], fp, tag="ad")
                nc.sync.dma_start(out=ad_t[:], in_=ad_dram[r0:r0 + 128, :])
                idx_t = eg_i.tile([128, icoln - icol0], i16, tag="idx")
                nc.sync.dma_start(out=idx_t[:], in_=idxbuf_d[:, icol0:icoln])
                G = eg_g.tile([128, d, W], fp, tag="G")
                for (v, c0, ncc, io) in bcalls:
                    nc.gpsimd.dma_gather(
                        G[:, c0:c0 + ncc, :], views[v],
                        idx_t[:, io - icol0:io - icol0 + ncc * 8],
                        128 * ncc, ni_regs[ncc], W, elem_step=W,
                        queue_num=qrr[0], single_packet=True,
                    )
                    qrr[0] = (qrr[0] + 1) % 4
                logit = eg_w.tile([128, d, H], fp, tag="logit")
                if H == 1:
                    nc.vector.tensor_scalar(
                        out=logit[:, :, 0:1], in0=G[:, :, 128:129],
                        scalar1=ad_t[:, 0:1], scalar2=None,
                        op0=mybir.AluOpType.add)
                else:
                    nc.vector.tensor_tensor(
                        out=logit[:, :, :], in0=G[:, :, 128:128 + H],
                        in1=vap(ad_t, [[0, d], [1, H]]), op=mybir.AluOpType.add)
                e1 = eg_w.tile([128, d, H], fp, tag="e1")
                fl = lambda t: t[:].rearrange("p k h -> p (k h)")
                nc.scalar.activation(fl(e1), fl(logit), mybir.ActivationFunctionType.Exp)
                wt = eg_w.tile([128, d, H], fp, tag="wt")
                nc.scalar.activation(fl(wt), fl(logit), mybir.ActivationFunctionType.Exp,
                                     scale=0.2)
                nc.vector.tensor_tensor(out=wt[:, :, :], in0=wt[:, :, :],
                                        in1=e1[:, :, :], op=mybir.AluOpType.max)
                # kill slack slots
                mslice = mask_sb[:, o:o + d]
                mbc = bass.AP(tensor=mslice.tensor, offset=mslice.offset,
                              ap=[mslice.ap[0], [1, d], [0, H]])
                nc.vector.tensor_tensor(
                    out=wt[:, :, :], in0=wt[:, :, :], in1=mbc,
                    op=mybir.AluOpType.mult)
                den = eg_s.tile([128, H], fp, tag="den")
                nc.vector.tensor_reduce(out=den[:, :], in_=vap(wt, [[1, H], [H, d]]),
                                        axis=mybir.AxisListType.X, op=mybir.AluOpType.add)
                nc.vector.tensor_scalar_add(den[:, :], den[:, :], 1e-30)
                rec = eg_s.tile([128, H], fp, tag="rec")
                nc.vector.reciprocal(rec[:, :], den[:, :])
                msg = eg_m.tile([128, d, 128], fp, tag="msg")
                nc.vector.tensor_tensor(
                    out=vap(msg, [[128, d], [Cc, H], [1, Cc]]),
                    in0=vap(G, [[W, d], [Cc, H], [1, Cc]]),
                    in1=vap(wt, [[H, d], [1, H], [0, Cc]]),
                    op=mybir.AluOpType.mult)
                num = eg_o.tile([128, 128], fp, tag="num")
                nc.vector.tensor_reduce(out=num[:, :], in_=vap(msg, [[1, 128], [128, d]]),
                                        axis=mybir.AxisListType.X, op=mybir.AluOpType.add)
                outt = eg_o.tile([128, 128], fp, tag="outt")
                if H == 1:
                    nc.vector.tensor_scalar_mul(outt[:, :], num[:, :], rec[:, 0:1])
                else:
                    nc.vector.tensor_tensor(
                        out=vap(outt, [[Cc, H], [1, Cc]]),
                        in0=vap(num, [[Cc, H], [1, Cc]]),
                        in1=vap(rec, [[1, H], [0, Cc]]),
                        op=mybir.AluOpType.mult)
                nc.sync.dma_start(out=out_dram[r0:r0 + 128, :], in_=outt[:, :])

        # ---------------- layer 1 ----------------
        for _rep in range(reps):
            matmul_phase(x_shard, wc1_sb, 136, patch1_d, 4, haug1_loc, 132, ad1_loc,
                         elu_in=False)
            tc.strict_bb_all_engine_barrier()
            nc.gpsimd.collective_compute(
                "AllGather", mybir.AluOpType.bypass,
                ins=[haug1_loc[:, :]], outs=[haug1_tab[:, :]],
                replica_groups=AG_GROUPS)
            tc.strict_bb_all_engine_barrier()
            if stages >= 2:
                edge_phase(haug1_tab, ad1_loc, cfg.H1, x2_loc, 132)
                tc.strict_bb_all_engine_barrier()
                pe_sync()
            if stages >= 3:
                matmul_phase(x2_loc, wc2_sb, 130, patch2_d, 1, haug2_loc, 129, ad2_loc,
                             elu_in=True)
                tc.strict_bb_all_engine_barrier()
                nc.gpsimd.collective_compute(
                    "AllGather", mybir.AluOpType.bypass,
                    ins=[haug2_loc[:, :]], outs=[haug2_tab[:, :]],
                    replica_groups=AG_GROUPS)
                tc.strict_bb_all_engine_barrier()
            if stages >= 4:
                edge_phase(haug2_tab, ad2_loc, 1, out_d, 129)

    if split:
        split_multi_waits(nc)
    lower_extended_insts(nc)
    return nc


# ---------------------------------------------------------------- entry point

def run(cfg, inputs, trace=False, reps=1):
    from concourse.bass_utils import run_bass_kernel_spmd

    x = np.asarray(inputs["x"], dtype=np.float32)
    edge_index = np.asarray(inputs["edge_index"]).astype(np.int64)
    perms, sched, idxbufs, masks = host_prep(cfg, edge_index)
    wcat1, wcat2 = make_wcats(
        cfg, np.asarray(inputs["W1"], np.float32), np.asarray(inputs["a_src1"], np.float32),
        np.asarray(inputs["a_dst1"], np.float32), np.asarray(inputs["W2"], np.float32),
        np.asarray(inputs["a_src2"], np.float32), np.asarray(inputs["a_dst2"], np.float32))
    in_maps = make_core_inputs(cfg, x, perms, idxbufs, masks, wcat1, wcat2,
                               np.asarray(inputs["b1"], np.float32))
    nc = build_bass(cfg, sched, reps=reps)
    res = run_bass_kernel_spmd(nc, in_maps, list(range(cfg.NCORES)), trace=trace)

    out = np.zeros((cfg.N, 128), np.float32)
    for c in range(cfg.NCORES):
        base = c * cfg.SH
        cnt = min(cfg.SH, cfg.N - base)
        perm = perms[c]
        valid = perm < cnt
        shard = res.results[c]["out"]
        out[base + perm[valid]] = shard[valid]
    out += np.asarray(inputs["b2"], np.float32)[None, :]
    return out, res


def kernel(**inputs) -> np.ndarray:
    out, _ = run(FULL, inputs, trace=False)
    return out
